# revision 35
# baseline (speedup 1.0000x reference)
"""Bass/Tile TRN2 kernel for the attention module:

    pre    = prev_hidden @ W1[:H] + b1                    [B, H]
    hidden = tanh(pre[:, None, :] + ann @ W1[H:])         [B, S, H]
    score  = hidden @ W2 (+ b2; softmax-invariant, drop)  [B, S]
    alpha  = softmax(score, axis=1)
    ctx    = alpha @ ann                                  [B, 1, A]

B=32, S=4096, A=H=512. Sharding: data-parallel over batch, 4 batches per
core on 8 cores. Single pass over S per batch with an unnormalized
online softmax (scores are bounded: |score| <= sum|W2|+|b2| ~ 11.4, so
exp never overflows in fp32 and no running-max is needed):

    w_s = exp(score_s);  Z = sum w_s;  ctx = (sum w_s * ann_s) / Z

Layouts: the s-dim matmul (ann @ W1a) contracts over the feature dim a,
so it needs ann with a on SBUF partitions (annT); the context matmul
contracts over s, so it needs natural ann. Host supplies both layouts in
bf16 (same total HBM bytes as one fp32 copy).
"""

import os

import numpy as np
import ml_dtypes

B = 32
S = 4096
A = 512
H = 512
NCORES = 8
BL = B // NCORES  # 4 batches per core
SC = 512          # s-chunk processed per inner iteration
NSC = S // SC     # 8

BF16 = ml_dtypes.bfloat16

_BUILT = None       # (nc,) cache — Bass module is reusable across calls
LAST_RESULT = None  # last BassKernelResults, for test harness introspection

LDW_DEDUP = False  # dropping LDWs breaks LDW<->MM pairing (verified wrong results)

# Stage selection for HW attribution profiling (all on for the real kernel)
STAGES = {"dma", "step2", "tanh", "score", "exp", "transpose", "ctx"}


def _build_bass(loop_n=None):
    """Build the Bass module. loop_n wraps the main s-loop in a For_i
    executed loop_n times — a timing amplifier (outputs then meaningless);
    loop_n=None builds the real single-pass kernel."""
    from contextlib import ExitStack, nullcontext

    import concourse.bass as bass
    import concourse.tile as tile
    from concourse import bacc, mybir
    from concourse.masks import make_identity

    bf16 = mybir.dt.bfloat16
    f32 = mybir.dt.float32
    Tanh = mybir.ActivationFunctionType.Tanh
    Exp = mybir.ActivationFunctionType.Exp

    nc = bacc.Bacc()

    annT_d = nc.dram_tensor("annT", [BL, A, S], bf16, kind="ExternalInput")
    annN_d = nc.dram_tensor("annN", [BL, S, A], bf16, kind="ExternalInput")
    w1a_d = nc.dram_tensor("w1a", [A, H], bf16, kind="ExternalInput")
    w1h_d = nc.dram_tensor("w1h", [H, H], bf16, kind="ExternalInput")
    b1_d = nc.dram_tensor("b1", [1, H], bf16, kind="ExternalInput")
    w2_d = nc.dram_tensor("w2", [H, 32], bf16, kind="ExternalInput")
    pvt_d = nc.dram_tensor("pvt", [H, BL], bf16, kind="ExternalInput")
    out_d = nc.dram_tensor("out", [BL, A], f32, kind="ExternalOutput")

    with tile.TileContext(nc) as tc, ExitStack() as ctx:
        singles = ctx.enter_context(tc.tile_pool(name="singles", bufs=1))
        annt_pool = ctx.enter_context(tc.tile_pool(name="annt", bufs=2))
        annn_pool = ctx.enter_context(tc.tile_pool(name="annn", bufs=2))
        th_pool = ctx.enter_context(tc.tile_pool(name="thp", bufs=2))
        w_pool = ctx.enter_context(tc.tile_pool(name="wp", bufs=2))
        psum2 = ctx.enter_context(
            tc.tile_pool(name="psum2", bufs=1, space="PSUM")
        )
        psum_wc = ctx.enter_context(
            tc.tile_pool(name="psumwc", bufs=2, space="PSUM")
        )
        psum1 = ctx.enter_context(
            tc.tile_pool(name="psum1", bufs=1, space="PSUM")
        )

        # ---- constants / weights in SBUF ----
        ident = singles.tile([128, 128], bf16)
        make_identity(nc, ident)

        w1a_sb = singles.tile([128, 4, H], bf16)  # (a%128, a//128, h)
        nc.sync.dma_start(
            out=w1a_sb, in_=w1a_d[:, :].rearrange("(ac p) h -> p ac h", p=128)
        )
        w1h_sb = singles.tile([128, 4, H], bf16)  # (hin%128, hin//128, h)
        nc.sync.dma_start(
            out=w1h_sb, in_=w1h_d[:, :].rearrange("(kc p) h -> p kc h", p=128)
        )
        b1_sb = singles.tile([1, H], bf16)
        nc.sync.dma_start(out=b1_sb, in_=b1_d[:, :])
        # W2 replicated x32 so score matmuls write a full 32-row col group
        w2_sb = singles.tile([128, 4, 32], bf16)  # (h%128, h//128, rep)
        nc.sync.dma_start(
            out=w2_sb, in_=w2_d[:, :].rearrange("(hc p) r -> p hc r", p=128)
        )
        pvt_sb = singles.tile([128, 4, BL], bf16)  # (hin%128, hin//128, b)
        nc.sync.dma_start(
            out=pvt_sb, in_=pvt_d[:, :].rearrange("(kc p) b -> p kc b", p=128)
        )
        ones_sb = singles.tile([1, BL], bf16)
        nc.vector.memset(ones_sb, 1.0)

        # ---- pre2T[h, b] = (prev @ W1h).T + b1 broadcast, in PSUM ----
        pre_ps = psum2.tile([128, 4, BL], f32, tag="score")
        for hc in range(4):
            for kc in range(4):
                nc.tensor.matmul(
                    pre_ps[:, hc, :],
                    lhsT=w1h_sb[:, kc, hc * 128:(hc + 1) * 128],
                    rhs=pvt_sb[:, kc, :],
                    start=(kc == 0),
                    stop=False,
                )
            # b1 contribution: rank-1 with ones row (K=1)
            nc.tensor.matmul(
                pre_ps[:, hc, :],
                lhsT=b1_sb[:, hc * 128:(hc + 1) * 128],
                rhs=ones_sb[:, :],
                start=False,
                stop=True,
            )
        pre_sb = singles.tile([128, 4, BL], f32)
        nc.scalar.copy(out=pre_sb, in_=pre_ps)

        # ---- main streaming loop over s-chunks ----
        z_sb = singles.tile([128, NSC], f32)
        ctx_ps = psum1.tile([128, A], f32, tag="ctx")

        outer = (
            tc.For_i(0, loop_n, 1) if loop_n is not None else nullcontext()
        )
        with outer:
            _main_body(
                nc, tc, mybir,
                annT_d, annN_d, w1a_sb, w2_sb, pre_sb, ident,
                annt_pool, annn_pool, th_pool, w_pool, psum2, psum_wc,
                z_sb, ctx_ps,
            )

        # ---- normalize and store ----
        out_sb = singles.tile([128, A], f32)
        if "exp" in STAGES and "ctx" in STAGES:
            z_tot = singles.tile([128, 1], f32)
            nc.vector.reduce_sum(
                out=z_tot, in_=z_sb, axis=mybir.AxisListType.X
            )
            z_rec = singles.tile([128, 1], f32)
            nc.vector.reciprocal(out=z_rec, in_=z_tot)
            nc.vector.tensor_scalar_mul(out_sb, ctx_ps[:, :], z_rec)
        else:
            nc.vector.memset(out_sb, 0.0)
        nc.sync.dma_start(out=out_d[:, :], in_=out_sb[0:128:32, :])

    if LDW_DEDUP:
        _dedup_ldweights(nc, mybir)
    nc.finalize()
    return nc


def _dedup_ldweights(nc, mybir):
    """Drop InstLdweights whose weights AP is identical to the previous
    (kept) InstLdweights with no different load in between; waits/updates
    are spliced onto the following instruction."""
    for f in nc.m.functions:
        for blk in f.blocks:
            insts = list(blk.instructions)
            keep = []
            last_key = None
            pending_sync = None
            for inst in insts:
                tn = type(inst).__name__
                if tn == "InstLdweights":
                    key = str(inst.ins[0])
                    if key == last_key:
                        si = inst.sync_info
                        if si is not None and (si.on_wait or si.on_update):
                            if pending_sync is None:
                                pending_sync = ([], [])
                            pending_sync[0].extend(si.on_wait)
                            pending_sync[1].extend(si.on_update)
                        continue  # drop it
                    last_key = key
                elif tn == "InstMatmult":
                    pass  # matmuls don't invalidate loaded weights
                else:
                    pass  # other-engine instrs in the block don't touch PE
                if pending_sync is not None:
                    si = inst.sync_info
                    ow = list(pending_sync[0])
                    ou = list(pending_sync[1])
                    if si is not None:
                        ow += list(si.on_wait)
                        ou += list(si.on_update)
                    inst.sync_info = mybir.SyncInfo(on_wait=ow, on_update=ou)
                    pending_sync = None
                keep.append(inst)
            if len(keep) != len(insts):
                blk.instructions = keep


def _main_body(
    nc, tc, mybir,
    annT_d, annN_d, w1a_sb, w2_sb, pre_sb, ident,
    annt_pool, annn_pool, th_pool, w_pool, psum2, psum_wc,
    z_sb, ctx_ps,
):
    bf16 = mybir.dt.bfloat16
    f32 = mybir.dt.float32
    Tanh = mybir.ActivationFunctionType.Tanh
    Exp = mybir.ActivationFunctionType.Exp

    # Batch-inner matmul ordering: 4 consecutive MMs share the stationary
    # weight block (weight reloads are the dominant per-MM cost), and the
    # transpose+ctx tail of chunk sc-1 is deferred so its exp/DVE deps are
    # resolved before the PE reaches it.
    pend = None
    for sc in range(NSC + 1):
        if sc < NSC:
            score_ps = psum2.tile([128, SC], f32, tag="score")
            at_tiles, an_tiles, th_tiles, thp_tiles = [], [], [], []
            for b in range(BL):
                at_sb = annt_pool.tile([128, 4, SC], bf16, tag=f"at{b}")
                if "dma" in STAGES:
                    nc.sync.dma_start(
                        out=at_sb,
                        in_=annT_d[b, :, sc * SC:(sc + 1) * SC].rearrange(
                            "(ac p) s -> p ac s", p=128
                        ),
                    )
                else:
                    nc.vector.memset(at_sb[:, 0, 0:1], 0.5)
                at_tiles.append(at_sb)
                an_sb = annn_pool.tile([128, 4, A], bf16, tag=f"an{b}")
                if "dma" in STAGES:
                    nc.sync.dma_start(
                        out=an_sb,
                        in_=annN_d[b, sc * SC:(sc + 1) * SC, :].rearrange(
                            "(sb p) a -> p sb a", p=128
                        ),
                    )
                else:
                    nc.vector.memset(an_sb[:, 0, 0:1], 0.5)
                an_tiles.append(an_sb)
                th_sb = th_pool.tile([128, 4, SC], bf16, tag=f"th{b}")
                if "step2" not in STAGES or "tanh" not in STAGES:
                    nc.vector.memset(th_sb[:, 0, 0:1], 0.5)
                th_tiles.append(th_sb)
                thp = psum2.tile([128, SC], f32, tag=f"thp{b}")
                thp_tiles.append(thp)

            if "step2" in STAGES:
                for hc in range(4):
                    for ac in range(4):
                        for b in range(BL):
                            nc.tensor.matmul(
                                thp_tiles[b][:, :],
                                lhsT=w1a_sb[:, ac, hc * 128:(hc + 1) * 128],
                                rhs=at_tiles[b][:, ac, :],
                                start=(ac == 0),
                                stop=(ac == 3),
                            )
                    if "tanh" in STAGES:
                        for b in range(BL):
                            nc.scalar.activation(
                                out=th_tiles[b][:, hc, :],
                                in_=thp_tiles[b][:, :],
                                func=Tanh,
                                bias=pre_sb[:, hc, b:b + 1],
                                scale=1.0,
                            )
            else:
                for b in range(BL):
                    nc.vector.memset(thp_tiles[b][:, 0:1], 0.5)

            if "score" in STAGES:
                for hc in range(4):
                    for b in range(BL):
                        nc.tensor.matmul(
                            score_ps[32 * b:32 * b + 32, :],
                            lhsT=w2_sb[:, hc, :],
                            rhs=th_tiles[b][:, hc, :],
                            start=(hc == 0),
                            stop=(hc == 3),
                            tile_position=(0, 32 * b),
                        )
            else:
                nc.vector.memset(score_ps[:, 0:1], 0.5)

            w_sb = w_pool.tile([128, SC], bf16, tag="w")
            if "exp" in STAGES:
                nc.scalar.activation(
                    out=w_sb,
                    in_=score_ps[:, :],
                    func=Exp,
                    accum_out=z_sb[:, sc:sc + 1],
                )
            else:
                nc.vector.memset(w_sb[:, 0:1], 0.5)
        else:
            w_sb = None
            an_tiles = None

        if pend is not None:
            p_w, p_an, p_sc = pend
            wcol_sb = w_pool.tile([128, 4, 128], bf16, tag="wcol")
            if "transpose" in STAGES:
                for st in range(4):
                    wc_ps = psum_wc.tile([128, 128], bf16, tag="wc")
                    nc.tensor.transpose(
                        wc_ps[:, :], p_w[:, st * 128:(st + 1) * 128],
                        ident[:, :],
                    )
                    nc.vector.tensor_copy(
                        out=wcol_sb[:, st, :], in_=wc_ps[:, :]
                    )
            else:
                nc.vector.memset(wcol_sb[:, 0, 0:1], 0.5)
            if "ctx" in STAGES:
                # st outer / b inner: consecutive MMs target disjoint
                # psum col groups -> they run concurrently on the PE
                for st in range(4):
                    for b in range(BL):
                        nc.tensor.matmul(
                            ctx_ps[32 * b:32 * b + 32, :],
                            lhsT=wcol_sb[:, st, 32 * b:32 * b + 32],
                            rhs=p_an[b][:, st, :],
                            start=(p_sc == 0 and st == 0),
                            stop=(p_sc == NSC - 1 and st == 3),
                            tile_position=(0, 32 * b),
                        )
        pend = (w_sb, an_tiles, sc) if sc < NSC else None


def _make_in_maps(prev_hidden_state, annotations, W1, b1, W2):
    prev_hidden_state = np.asarray(prev_hidden_state, dtype=np.float32)
    annotations = np.asarray(annotations, dtype=np.float32)
    W1 = np.asarray(W1, dtype=np.float32)
    b1 = np.asarray(b1, dtype=np.float32)
    W2 = np.asarray(W2, dtype=np.float32)

    annN = annotations.astype(BF16)
    annT = np.ascontiguousarray(annotations.transpose(0, 2, 1)).astype(BF16)
    w1h = np.ascontiguousarray(W1[:H]).astype(BF16)
    w1a = np.ascontiguousarray(W1[H:]).astype(BF16)
    b1r = b1.reshape(1, H).astype(BF16)
    w2c = np.ascontiguousarray(np.tile(W2.reshape(H, 1), (1, 32))).astype(BF16)
    pvt = np.ascontiguousarray(prev_hidden_state.T).astype(BF16)  # [H, B]

    in_maps = []
    for c in range(NCORES):
        sl = slice(c * BL, (c + 1) * BL)
        in_maps.append(
            {
                "annT": np.ascontiguousarray(annT[sl]),
                "annN": np.ascontiguousarray(annN[sl]),
                "w1a": w1a,
                "w1h": w1h,
                "b1": b1r,
                "w2": w2c,
                "pvt": np.ascontiguousarray(pvt[:, sl]),
            }
        )
    return in_maps


def kernel(prev_hidden_state, annotations, W1, b1, W2, b2, **_unused):
    global _BUILT, LAST_RESULT
    from concourse import bass_utils

    # b2 shifts every score equally; softmax is shift-invariant -> ignored.
    in_maps = _make_in_maps(prev_hidden_state, annotations, W1, b1, W2)

    if _BUILT is None:
        _BUILT = _build_bass()
    nc = _BUILT

    trace = bool(int(os.environ.get("KERNEL_TRACE", "0")))
    if not trace:
        # the NTFF trace path needs antenv.axon_hooks, absent in this
        # client -- make sure an ambient BASS_TRACE can't select it
        os.environ.setdefault("BASS_NEVER_TRACE", "1")
    res = bass_utils.run_bass_kernel_spmd(
        nc, in_maps, core_ids=list(range(NCORES)), trace=trace
    )
    LAST_RESULT = res
    out = np.concatenate([r["out"] for r in res.results], axis=0)  # [B, A]
    return out[:, None, :].astype(np.float32)


# revision 37
# speedup vs baseline: 1.0004x; 1.0004x over previous
"""Bass/Tile TRN2 kernel for the attention module:

    pre    = prev_hidden @ W1[:H] + b1                    [B, H]
    hidden = tanh(pre[:, None, :] + ann @ W1[H:])         [B, S, H]
    score  = hidden @ W2 (+ b2; softmax-invariant, drop)  [B, S]
    alpha  = softmax(score, axis=1)
    ctx    = alpha @ ann                                  [B, 1, A]

B=32, S=4096, A=H=512. Sharding: data-parallel over batch, 4 batches per
core on 8 cores. Single pass over S per batch with an unnormalized
online softmax (scores are bounded: |score| <= sum|W2|+|b2| ~ 11.4, so
exp never overflows in fp32 and no running-max is needed):

    w_s = exp(score_s);  Z = sum w_s;  ctx = (sum w_s * ann_s) / Z

Layouts: the s-dim matmul (ann @ W1a) contracts over the feature dim a,
so it needs ann with a on SBUF partitions (annT); the context matmul
contracts over s, so it needs natural ann. Host supplies both layouts in
bf16 (same total HBM bytes as one fp32 copy).
"""

import os

import numpy as np
import ml_dtypes

B = 32
S = 4096
A = 512
H = 512
NCORES = 8
BL = B // NCORES  # 4 batches per core
SC = 512          # s-chunk processed per inner iteration
NSC = S // SC     # 8

BF16 = ml_dtypes.bfloat16

_BUILT = None       # (nc,) cache — Bass module is reusable across calls
LAST_RESULT = None  # last BassKernelResults, for test harness introspection

LDW_DEDUP = False  # dropping LDWs breaks LDW<->MM pairing (verified wrong results)

# Stage selection for HW attribution profiling (all on for the real kernel)
STAGES = {"dma", "step2", "tanh", "score", "exp", "transpose", "ctx"}


def _build_bass(loop_n=None):
    """Build the Bass module. loop_n wraps the main s-loop in a For_i
    executed loop_n times — a timing amplifier (outputs then meaningless);
    loop_n=None builds the real single-pass kernel."""
    from contextlib import ExitStack, nullcontext

    import concourse.bass as bass
    import concourse.tile as tile
    from concourse import bacc, mybir
    from concourse.masks import make_identity

    bf16 = mybir.dt.bfloat16
    f32 = mybir.dt.float32
    Tanh = mybir.ActivationFunctionType.Tanh
    Exp = mybir.ActivationFunctionType.Exp

    nc = bacc.Bacc()

    annT_d = nc.dram_tensor("annT", [BL, A, S], bf16, kind="ExternalInput")
    annN_d = nc.dram_tensor("annN", [BL, S, A], bf16, kind="ExternalInput")
    w1a_d = nc.dram_tensor("w1a", [A, H], bf16, kind="ExternalInput")
    w1h_d = nc.dram_tensor("w1h", [H, H], bf16, kind="ExternalInput")
    b1_d = nc.dram_tensor("b1", [1, H], bf16, kind="ExternalInput")
    w2_d = nc.dram_tensor("w2", [H, 32], bf16, kind="ExternalInput")
    pvt_d = nc.dram_tensor("pvt", [H, BL], bf16, kind="ExternalInput")
    out_d = nc.dram_tensor("out", [BL, A], f32, kind="ExternalOutput")

    with tile.TileContext(nc) as tc, ExitStack() as ctx:
        singles = ctx.enter_context(tc.tile_pool(name="singles", bufs=1))
        annt_pool = ctx.enter_context(tc.tile_pool(name="annt", bufs=4))
        annn_pool = ctx.enter_context(tc.tile_pool(name="annn", bufs=3))
        th_pool = ctx.enter_context(tc.tile_pool(name="thp", bufs=3))
        w_pool = ctx.enter_context(tc.tile_pool(name="wp", bufs=3))
        psum2 = ctx.enter_context(
            tc.tile_pool(name="psum2", bufs=1, space="PSUM")
        )
        psum_wc = ctx.enter_context(
            tc.tile_pool(name="psumwc", bufs=2, space="PSUM")
        )
        psum1 = ctx.enter_context(
            tc.tile_pool(name="psum1", bufs=1, space="PSUM")
        )

        # ---- constants / weights in SBUF ----
        ident = singles.tile([128, 128], bf16)
        make_identity(nc, ident)

        w1a_sb = singles.tile([128, 4, H], bf16)  # (a%128, a//128, h)
        nc.sync.dma_start(
            out=w1a_sb, in_=w1a_d[:, :].rearrange("(ac p) h -> p ac h", p=128)
        )
        w1h_sb = singles.tile([128, 4, H], bf16)  # (hin%128, hin//128, h)
        nc.sync.dma_start(
            out=w1h_sb, in_=w1h_d[:, :].rearrange("(kc p) h -> p kc h", p=128)
        )
        b1_sb = singles.tile([1, H], bf16)
        nc.sync.dma_start(out=b1_sb, in_=b1_d[:, :])
        # W2 replicated x32 so score matmuls write a full 32-row col group
        w2_sb = singles.tile([128, 4, 32], bf16)  # (h%128, h//128, rep)
        nc.sync.dma_start(
            out=w2_sb, in_=w2_d[:, :].rearrange("(hc p) r -> p hc r", p=128)
        )
        pvt_sb = singles.tile([128, 4, BL], bf16)  # (hin%128, hin//128, b)
        nc.sync.dma_start(
            out=pvt_sb, in_=pvt_d[:, :].rearrange("(kc p) b -> p kc b", p=128)
        )
        ones_sb = singles.tile([1, BL], bf16)
        nc.vector.memset(ones_sb, 1.0)

        # ---- pre2T[h, b] = (prev @ W1h).T + b1 broadcast, in PSUM ----
        pre_ps = psum2.tile([128, 4, BL], f32, tag="score")
        for hc in range(4):
            for kc in range(4):
                nc.tensor.matmul(
                    pre_ps[:, hc, :],
                    lhsT=w1h_sb[:, kc, hc * 128:(hc + 1) * 128],
                    rhs=pvt_sb[:, kc, :],
                    start=(kc == 0),
                    stop=False,
                )
            # b1 contribution: rank-1 with ones row (K=1)
            nc.tensor.matmul(
                pre_ps[:, hc, :],
                lhsT=b1_sb[:, hc * 128:(hc + 1) * 128],
                rhs=ones_sb[:, :],
                start=False,
                stop=True,
            )
        pre_sb = singles.tile([128, 4, BL], f32)
        nc.scalar.copy(out=pre_sb, in_=pre_ps)

        # ---- main streaming loop over s-chunks ----
        z_sb = singles.tile([128, NSC], f32)
        ctx_ps = psum1.tile([128, A], f32, tag="ctx")

        outer = (
            tc.For_i(0, loop_n, 1) if loop_n is not None else nullcontext()
        )
        with outer:
            _main_body(
                nc, tc, mybir,
                annT_d, annN_d, w1a_sb, w2_sb, pre_sb, ident,
                annt_pool, annn_pool, th_pool, w_pool, psum2, psum_wc,
                z_sb, ctx_ps,
            )

        # ---- normalize and store ----
        out_sb = singles.tile([128, A], f32)
        if "exp" in STAGES and "ctx" in STAGES:
            z_tot = singles.tile([128, 1], f32)
            nc.vector.reduce_sum(
                out=z_tot, in_=z_sb, axis=mybir.AxisListType.X
            )
            z_rec = singles.tile([128, 1], f32)
            nc.vector.reciprocal(out=z_rec, in_=z_tot)
            nc.vector.tensor_scalar_mul(out_sb, ctx_ps[:, :], z_rec)
        else:
            nc.vector.memset(out_sb, 0.0)
        nc.sync.dma_start(out=out_d[:, :], in_=out_sb[0:128:32, :])

    if LDW_DEDUP:
        _dedup_ldweights(nc, mybir)
    nc.finalize()
    return nc


def _dedup_ldweights(nc, mybir):
    """Drop InstLdweights whose weights AP is identical to the previous
    (kept) InstLdweights with no different load in between; waits/updates
    are spliced onto the following instruction."""
    for f in nc.m.functions:
        for blk in f.blocks:
            insts = list(blk.instructions)
            keep = []
            last_key = None
            pending_sync = None
            for inst in insts:
                tn = type(inst).__name__
                if tn == "InstLdweights":
                    key = str(inst.ins[0])
                    if key == last_key:
                        si = inst.sync_info
                        if si is not None and (si.on_wait or si.on_update):
                            if pending_sync is None:
                                pending_sync = ([], [])
                            pending_sync[0].extend(si.on_wait)
                            pending_sync[1].extend(si.on_update)
                        continue  # drop it
                    last_key = key
                elif tn == "InstMatmult":
                    pass  # matmuls don't invalidate loaded weights
                else:
                    pass  # other-engine instrs in the block don't touch PE
                if pending_sync is not None:
                    si = inst.sync_info
                    ow = list(pending_sync[0])
                    ou = list(pending_sync[1])
                    if si is not None:
                        ow += list(si.on_wait)
                        ou += list(si.on_update)
                    inst.sync_info = mybir.SyncInfo(on_wait=ow, on_update=ou)
                    pending_sync = None
                keep.append(inst)
            if len(keep) != len(insts):
                blk.instructions = keep


def _main_body(
    nc, tc, mybir,
    annT_d, annN_d, w1a_sb, w2_sb, pre_sb, ident,
    annt_pool, annn_pool, th_pool, w_pool, psum2, psum_wc,
    z_sb, ctx_ps,
):
    bf16 = mybir.dt.bfloat16
    f32 = mybir.dt.float32
    Tanh = mybir.ActivationFunctionType.Tanh
    Exp = mybir.ActivationFunctionType.Exp

    # Batch-inner matmul ordering: 4 consecutive MMs share the stationary
    # weight block (weight reloads are the dominant per-MM cost), and the
    # transpose+ctx tail of chunk sc-1 is deferred so its exp/DVE deps are
    # resolved before the PE reaches it.
    pend = None
    for sc in range(NSC + 1):
        if sc < NSC:
            score_ps = psum2.tile([128, SC], f32, tag="score")
            at_tiles, an_tiles, th_tiles, thp_tiles = [], [], [], []
            for b in range(BL):
                at_sb = annt_pool.tile([128, 4, SC], bf16, tag=f"at{b}")
                if "dma" in STAGES:
                    nc.sync.dma_start(
                        out=at_sb,
                        in_=annT_d[b, :, sc * SC:(sc + 1) * SC].rearrange(
                            "(ac p) s -> p ac s", p=128
                        ),
                    )
                else:
                    nc.vector.memset(at_sb[:, 0, 0:1], 0.5)
                at_tiles.append(at_sb)
                an_sb = annn_pool.tile([128, 4, A], bf16, tag=f"an{b}")
                if "dma" in STAGES:
                    nc.sync.dma_start(
                        out=an_sb,
                        in_=annN_d[b, sc * SC:(sc + 1) * SC, :].rearrange(
                            "(sb p) a -> p sb a", p=128
                        ),
                    )
                else:
                    nc.vector.memset(an_sb[:, 0, 0:1], 0.5)
                an_tiles.append(an_sb)
                th_sb = th_pool.tile([128, 4, SC], bf16, tag=f"th{b}")
                if "step2" not in STAGES or "tanh" not in STAGES:
                    nc.vector.memset(th_sb[:, 0, 0:1], 0.5)
                th_tiles.append(th_sb)
                thp = psum2.tile([128, SC], f32, tag=f"thp{b}")
                thp_tiles.append(thp)

            if "step2" in STAGES:
                for hc in range(4):
                    for ac in range(4):
                        for b in range(BL):
                            nc.tensor.matmul(
                                thp_tiles[b][:, :],
                                lhsT=w1a_sb[:, ac, hc * 128:(hc + 1) * 128],
                                rhs=at_tiles[b][:, ac, :],
                                start=(ac == 0),
                                stop=(ac == 3),
                            )
                    if "tanh" in STAGES:
                        for b in range(BL):
                            nc.scalar.activation(
                                out=th_tiles[b][:, hc, :],
                                in_=thp_tiles[b][:, :],
                                func=Tanh,
                                bias=pre_sb[:, hc, b:b + 1],
                                scale=1.0,
                            )
            else:
                for b in range(BL):
                    nc.vector.memset(thp_tiles[b][:, 0:1], 0.5)

            if "score" in STAGES:
                for hc in range(4):
                    for b in range(BL):
                        nc.tensor.matmul(
                            score_ps[32 * b:32 * b + 32, :],
                            lhsT=w2_sb[:, hc, :],
                            rhs=th_tiles[b][:, hc, :],
                            start=(hc == 0),
                            stop=(hc == 3),
                            tile_position=(0, 32 * b),
                        )
            else:
                nc.vector.memset(score_ps[:, 0:1], 0.5)

            w_sb = w_pool.tile([128, SC], bf16, tag="w")
            if "exp" in STAGES:
                nc.scalar.activation(
                    out=w_sb,
                    in_=score_ps[:, :],
                    func=Exp,
                    accum_out=z_sb[:, sc:sc + 1],
                )
            else:
                nc.vector.memset(w_sb[:, 0:1], 0.5)
        else:
            w_sb = None
            an_tiles = None

        if pend is not None:
            p_w, p_an, p_sc = pend
            wcol_sb = w_pool.tile([128, 4, 128], bf16, tag="wcol")
            if "transpose" in STAGES:
                for st in range(4):
                    wc_ps = psum_wc.tile([128, 128], bf16, tag="wc")
                    nc.tensor.transpose(
                        wc_ps[:, :], p_w[:, st * 128:(st + 1) * 128],
                        ident[:, :],
                    )
                    nc.vector.tensor_copy(
                        out=wcol_sb[:, st, :], in_=wc_ps[:, :]
                    )
            else:
                nc.vector.memset(wcol_sb[:, 0, 0:1], 0.5)
            if "ctx" in STAGES:
                # st outer / b inner: consecutive MMs target disjoint
                # psum col groups -> they run concurrently on the PE
                for st in range(4):
                    for b in range(BL):
                        nc.tensor.matmul(
                            ctx_ps[32 * b:32 * b + 32, :],
                            lhsT=wcol_sb[:, st, 32 * b:32 * b + 32],
                            rhs=p_an[b][:, st, :],
                            start=(p_sc == 0 and st == 0),
                            stop=(p_sc == NSC - 1 and st == 3),
                            tile_position=(0, 32 * b),
                        )
        pend = (w_sb, an_tiles, sc) if sc < NSC else None


def _make_in_maps(prev_hidden_state, annotations, W1, b1, W2):
    prev_hidden_state = np.asarray(prev_hidden_state, dtype=np.float32)
    annotations = np.asarray(annotations, dtype=np.float32)
    W1 = np.asarray(W1, dtype=np.float32)
    b1 = np.asarray(b1, dtype=np.float32)
    W2 = np.asarray(W2, dtype=np.float32)

    annN = annotations.astype(BF16)
    annT = np.ascontiguousarray(annotations.transpose(0, 2, 1)).astype(BF16)
    w1h = np.ascontiguousarray(W1[:H]).astype(BF16)
    w1a = np.ascontiguousarray(W1[H:]).astype(BF16)
    b1r = b1.reshape(1, H).astype(BF16)
    w2c = np.ascontiguousarray(np.tile(W2.reshape(H, 1), (1, 32))).astype(BF16)
    pvt = np.ascontiguousarray(prev_hidden_state.T).astype(BF16)  # [H, B]

    in_maps = []
    for c in range(NCORES):
        sl = slice(c * BL, (c + 1) * BL)
        in_maps.append(
            {
                "annT": np.ascontiguousarray(annT[sl]),
                "annN": np.ascontiguousarray(annN[sl]),
                "w1a": w1a,
                "w1h": w1h,
                "b1": b1r,
                "w2": w2c,
                "pvt": np.ascontiguousarray(pvt[:, sl]),
            }
        )
    return in_maps


def kernel(prev_hidden_state, annotations, W1, b1, W2, b2, **_unused):
    global _BUILT, LAST_RESULT
    from concourse import bass_utils

    # b2 shifts every score equally; softmax is shift-invariant -> ignored.
    in_maps = _make_in_maps(prev_hidden_state, annotations, W1, b1, W2)

    if _BUILT is None:
        _BUILT = _build_bass()
    nc = _BUILT

    trace = bool(int(os.environ.get("KERNEL_TRACE", "0")))
    if not trace:
        # the NTFF trace path needs antenv.axon_hooks, absent in this
        # client -- make sure an ambient BASS_TRACE can't select it
        os.environ.setdefault("BASS_NEVER_TRACE", "1")
    res = bass_utils.run_bass_kernel_spmd(
        nc, in_maps, core_ids=list(range(NCORES)), trace=trace
    )
    LAST_RESULT = res
    out = np.concatenate([r["out"] for r in res.results], axis=0)  # [B, A]
    return out[:, None, :].astype(np.float32)
